# revision 21
# baseline (speedup 1.0000x reference)
"""Multi-head causal attention (B=4, C=2048, E=1024, H=16, D=64) on 8 trn2 cores.

Sharding: core i = (batch b=i//2, head-group g=i%2).  Each core computes its
batch's attention for 8 heads (512 features) and a partial output projection;
the host sums the two partials per batch (W_o split row-wise).

Per-core kernel (all matmuls float32r: full PE rate at N>=256, FP22 operands):
  phase 1: V = x @ Wv_g            -> [tok, 8 heads x (64 feat + ones col)]
           QT/KT per head-pair     -> [128 feat, 2048 tok]   (x.T pre-done on host)
  phase 2: per (head-pair, q-chunk 512, k-block 128):
           S^T = K^T.T @ Q^T       (row-tiled pair, K=64 contraction x 2 heads)
           W^T = exp(S^T / 8)      (one ACT over both heads' psum banks)
           diagonal causal mask    (DVE multiply with host-provided mask)
           hid/rowsum = [V|1].T @ W^T  (M=65 accumulating over k-blocks)
           normalize by 1/rowsum -> hiddenT staged to DRAM
  phase 3: out = hiddenT.T @ Wo_g  (K=512 contraction via 4 chained matmuls)
"""

import numpy as np

import concourse.bass as bass
import concourse.mybir as mybir
import concourse.tile as tile
from concourse.vector_clock import ScopedClock

B, C, E = 4, 2048, 1024
H, D = 16, 64
N_CORES = 8
GF = 512          # features per head-group (8 heads x 64)
HP = 4            # head-pairs per group
QC = 512          # q-chunk width
KB = 128          # k-block width
NQC = C // QC     # 4
NKB = C // KB     # 16
NE = E // 128     # 8 contraction tiles over E
F32 = mybir.dt.float32
F32R = mybir.dt.float32r

_CACHED_NC = None


class PatchedTC(tile.TileContext):
    """This walrus build caps sync waits per instruction (1 for CTRL, ~2 for
    compute ISA structs).  Hoist excess waits onto same-engine NOPs emitted
    just before the instruction (engine streams execute in order, so the
    semantics are identical), and split the end-of-kernel drain's waits
    across single-wait drain instructions."""

    WAIT_CAP = 1

    def _commit_instruction(self, inst, lazy_reg_writes=True):
        si = getattr(inst, "sync_info", None)
        if (
            si is not None
            and len(si.on_wait) > self.WAIT_CAP
            and getattr(inst, "engine", mybir.EngineType.Unassigned)
            != mybir.EngineType.Unassigned
        ):
            waits = list(si.on_wait)
            keep = waits[: self.WAIT_CAP]
            extra = waits[self.WAIT_CAP :]
            si.on_wait[:] = keep
            for w in extra:
                nop = mybir.InstNoOp(
                    name=f"I-nw{self.nc.next_id()}",
                    engine=inst.engine,
                    bass_nofuse=True,
                    sync_info=mybir.SyncInfo(on_wait=[w], on_update=[]),
                )
                super()._commit_instruction(nop, lazy_reg_writes=False)
        return super()._commit_instruction(inst, lazy_reg_writes)

    def _drain_and_barrier(self, tick_clock, wait_clock):
        carrier = self.nc.sync.drain()
        wait_clock.add_sem_waits(
            carrier.ins, ScopedClock({None: tick_clock.global_clock})
        )
        si = carrier.ins.sync_info
        waits = list(si.on_wait) if si is not None else []
        if len(waits) > 1:
            si.on_wait[:] = waits[:1]
            for w in waits[1:]:
                extra = self.nc.sync.drain()
                extra.ins.sync_info = mybir.SyncInfo(on_wait=[w], on_update=[])
        self.nc.all_engine_barrier()
        assert self.sems is not None
        popped = self.nc._tile_sem_poison_stack.pop()
        assert popped is self._sem_poison
        self.nc.clear_and_free_semaphores(list(self.sems.allocated().values()))
        self.nc.all_engine_barrier()


def build_nc():
    nc = bass.Bass("TRN2", target_bir_lowering=False)
    xT = nc.declare_dram_parameter("xT", [E, C], F32, isOutput=False)
    Wq = nc.declare_dram_parameter("Wq", [E, GF], F32, isOutput=False)
    Wk = nc.declare_dram_parameter("Wk", [E, GF], F32, isOutput=False)
    Wv = nc.declare_dram_parameter("Wv", [E, GF], F32, isOutput=False)
    Wo = nc.declare_dram_parameter("Wo", [GF, E], F32, isOutput=False)
    msk = nc.declare_dram_parameter("mask", [128, 4 * QC], mybir.dt.bfloat16, isOutput=False)
    out = nc.declare_dram_parameter("out", [C, E], F32, isOutput=True)

    xT_t = xT.ap().rearrange("(po pi) f -> pi po f", pi=128)    # [128, 8, C]
    Wq_t = Wq.ap().rearrange("(po pi) f -> pi po f", pi=128)    # [128, 8, GF]
    Wk_t = Wk.ap().rearrange("(po pi) f -> pi po f", pi=128)
    Wv_t = Wv.ap().rearrange("(po pi) f -> pi po f", pi=128)
    Wo_t = Wo.ap().rearrange("(po pi) f -> pi po f", pi=128)    # [128, 4, E]

    with PatchedTC(nc) as tc:
        import contextlib

        with contextlib.ExitStack() as ctx:
            consts = ctx.enter_context(tc.tile_pool(name="consts", bufs=1))
            dram = ctx.enter_context(tc.tile_pool(name="dram", bufs=1, space="DRAM"))
            ppsum = ctx.enter_context(tc.tile_pool(name="ppsum", bufs=2, space="PSUM"))

            mask_sb = consts.tile([128, 4 * QC], mybir.dt.bfloat16)
            nc.sync.dma_start(mask_sb[:], msk.ap())

            ctx2 = ctx.enter_context(contextlib.ExitStack())
            xpool = ctx2.enter_context(tc.tile_pool(name="xpool", bufs=1))
            vpool = ctx2.enter_context(tc.tile_pool(name="vpool", bufs=1))

            xT_sb = xpool.tile([128, NE, C], F32R)
            for e in range(NE):
                nc.sync.dma_start(xT_sb[:, e, :], xT_t[:, e, :].bitcast(F32R))

            # ---- phase 1a: V for all 8 heads, ones column appended per head
            with tc.tile_pool(name="wvpool", bufs=1) as wvpool:
                wv_sb = wvpool.tile([128, NE, GF], F32R)
                nc.sync.dma_start(wv_sb[:], Wv_t[:].bitcast(F32R))
                v_sb = vpool.tile([128, NKB, 2 * GF], F32R)  # [tok, kb, h*(64V|64ones)]
                nc.any.memset(v_sb[:].bitcast(F32), 1.0)
                for t in range(NKB):
                    pv = ppsum.tile([128, GF], F32, tag="ppsum")
                    for e in range(NE):
                        nc.tensor.matmul(
                            pv[:],
                            lhsT=xT_sb[:, e, t * 128 : (t + 1) * 128],
                            rhs=wv_sb[:, e, :],
                            start=(e == 0),
                            stop=(e == NE - 1),
                        )
                    dst = v_sb[:, t, :].rearrange("p (h u) -> p h u", u=128)[:, :, 0:64]
                    nc.vector.tensor_copy(dst, pv[:].rearrange("p (h u) -> p h u", u=64))

            # ---- phases 1b + 2: per head-pair projections + attention
            qkpool = ctx2.enter_context(tc.tile_pool(name="qkpool", bufs=2))
            wpool = ctx2.enter_context(tc.tile_pool(name="wpool", bufs=1))
            stpool = ctx2.enter_context(tc.tile_pool(name="stpsum", bufs=2, space="PSUM"))
            hidpool = ctx2.enter_context(tc.tile_pool(name="hidpsum", bufs=1, space="PSUM"))
            wtpool = ctx2.enter_context(tc.tile_pool(name="wtpool", bufs=2))
            napool = ctx2.enter_context(tc.tile_pool(name="napool", bufs=2))
            hidT_dram = dram.tile([HP, 128, C], F32R)

            for hp in range(HP):
                wq_sb = wpool.tile([128, NE, 128], F32R, tag="wq")
                wk_sb = wpool.tile([128, NE, 128], F32R, tag="wk")
                nc.sync.dma_start(wq_sb[:], Wq_t[:, :, hp * 128 : (hp + 1) * 128].bitcast(F32R))
                nc.sync.dma_start(wk_sb[:], Wk_t[:, :, hp * 128 : (hp + 1) * 128].bitcast(F32R))
                qt = qkpool.tile([128, C], F32R, tag="qt")
                kt = qkpool.tile([128, C], F32R, tag="kt")
                for n in range(NQC):
                    pq = ppsum.tile([128, QC], F32, tag="ppsum")
                    for e in range(NE):
                        nc.tensor.matmul(
                            pq[:],
                            lhsT=wq_sb[:, e, :],
                            rhs=xT_sb[:, e, n * QC : (n + 1) * QC],
                            start=(e == 0),
                            stop=(e == NE - 1),
                        )
                    nc.vector.tensor_copy(qt[:, n * QC : (n + 1) * QC], pq[:])
                    pk = ppsum.tile([128, QC], F32, tag="ppsum")
                    for e in range(NE):
                        nc.tensor.matmul(
                            pk[:],
                            lhsT=wk_sb[:, e, :],
                            rhs=xT_sb[:, e, n * QC : (n + 1) * QC],
                            start=(e == 0),
                            stop=(e == NE - 1),
                        )
                    nc.vector.tensor_copy(kt[:, n * QC : (n + 1) * QC], pk[:])

                for qc in range(NQC):
                    nkb = 4 * qc + 4
                    hidA = hidpool.tile([128, QC], F32, tag="hidA")
                    hidB = hidpool.tile([128, QC], F32, tag="hidB")
                    for kb in range(nkb):
                        st = stpool.tile([128, 2 * QC], F32, tag="st")
                        nc.tensor.matmul(
                            st[:, 0:QC],
                            lhsT=kt[0:64, kb * KB : (kb + 1) * KB],
                            rhs=qt[0:64, qc * QC : (qc + 1) * QC],
                            start=True,
                            stop=True,
                        )
                        nc.tensor.matmul(
                            st[:, QC : 2 * QC],
                            lhsT=kt[64:128, kb * KB : (kb + 1) * KB],
                            rhs=qt[64:128, qc * QC : (qc + 1) * QC],
                            start=True,
                            stop=True,
                        )
                        wt = wtpool.tile([128, 2 * QC], F32R, tag="wt")
                        nc.scalar.activation(
                            wt[:], st[:], mybir.ActivationFunctionType.Exp, scale=0.125
                        )
                        dr = kb - (nkb - 4)
                        if dr >= 0:
                            nc.vector.tensor_tensor(
                                wt[:].rearrange("p (a b) -> p a b", a=2),
                                wt[:].rearrange("p (a b) -> p a b", a=2),
                                mask_sb[:, None, dr * QC : (dr + 1) * QC].to_broadcast(
                                    (128, 2, QC)
                                ),
                                mybir.AluOpType.mult,
                            )
                        # hidden rows 0:64; rowsum replicated on rows 64:128
                        # (ones columns embedded in v_sb)
                        nc.tensor.matmul(
                            hidA[:],
                            lhsT=v_sb[:, kb, 2 * hp * 128 : (2 * hp + 1) * 128],
                            rhs=wt[:, 0:QC],
                            start=(kb == 0),
                            stop=(kb == nkb - 1),
                        )
                        nc.tensor.matmul(
                            hidB[:],
                            lhsT=v_sb[:, kb, (2 * hp + 1) * 128 : (2 * hp + 2) * 128],
                            rhs=wt[:, QC : 2 * QC],
                            start=(kb == 0),
                            stop=(kb == nkb - 1),
                        )
                    recA = napool.tile([64, QC], F32, tag="rec")
                    recB = napool.tile([64, QC], F32, tag="rec")
                    nc.vector.reciprocal(recA[:], hidA[64:128, :])
                    nc.vector.reciprocal(recB[:], hidB[64:128, :])
                    stage = napool.tile([128, QC], F32R, tag="stage")
                    nc.vector.tensor_tensor(
                        stage[0:64, :], hidA[0:64, :], recA[:], mybir.AluOpType.mult
                    )
                    nc.vector.tensor_tensor(
                        stage[64:128, :], hidB[0:64, :], recB[:], mybir.AluOpType.mult
                    )
                    nc.sync.dma_start(
                        hidT_dram[hp, :, qc * QC : (qc + 1) * QC], stage[:]
                    )

            # ---- phase 3: out projection, contracting all 512 group features
            ctx2.close()
            with tc.tile_pool(name="opool", bufs=1) as opool, tc.tile_pool(
                name="ostage", bufs=3
            ) as ostage:
                wo_sb = opool.tile([128, HP, E], F32R)
                nc.sync.dma_start(wo_sb[:], Wo_t[:].bitcast(F32R))
                hf = opool.tile([128, HP, C], F32R)
                for f in range(HP):
                    nc.sync.dma_start(hf[:, f, :], hidT_dram[f, :, :])
                for qb in range(C // 128):
                    for ec in range(E // QC):
                        po = ppsum.tile([128, QC], F32, tag="ppsum")
                        for f in range(HP):
                            nc.tensor.matmul(
                                po[:],
                                lhsT=hf[:, f, qb * 128 : (qb + 1) * 128],
                                rhs=wo_sb[:, f, ec * QC : (ec + 1) * QC],
                                start=(f == 0),
                                stop=(f == HP - 1),
                            )
                        so = ostage.tile([128, QC], F32, tag="so")
                        nc.vector.tensor_copy(so[:], po[:])
                        nc.sync.dma_start(
                            out.ap()[qb * 128 : (qb + 1) * 128, ec * QC : (ec + 1) * QC],
                            so[:],
                        )
    return nc


def _make_mask():
    import ml_dtypes

    m = np.zeros((128, 4, QC), dtype=np.float32)
    for rr in range(4):
        kk = np.arange(128)[:, None]
        qq = np.arange(QC)[None, :]
        m[:, rr, :] = (128 * rr + kk <= qq).astype(np.float32)
    return np.ascontiguousarray(m.reshape(128, 4 * QC)).astype(ml_dtypes.bfloat16)


def kernel(x, W_q, W_k, W_v, W_o):
    global _CACHED_NC
    from concourse.bass_utils import run_bass_kernel_spmd

    if _CACHED_NC is None:
        _CACHED_NC = build_nc()
    nc = _CACHED_NC

    mask = _make_mask()
    in_maps = []
    for i in range(N_CORES):
        b, g = i // 2, i % 2
        in_maps.append(
            {
                "xT": np.ascontiguousarray(np.asarray(x)[b].T),
                "Wq": np.ascontiguousarray(np.asarray(W_q)[:, g * GF : (g + 1) * GF]),
                "Wk": np.ascontiguousarray(np.asarray(W_k)[:, g * GF : (g + 1) * GF]),
                "Wv": np.ascontiguousarray(np.asarray(W_v)[:, g * GF : (g + 1) * GF]),
                "Wo": np.ascontiguousarray(np.asarray(W_o)[g * GF : (g + 1) * GF, :]),
                "mask": mask,
            }
        )
    res = run_bass_kernel_spmd(nc, in_maps, core_ids=list(range(N_CORES)))
    out = np.empty((B, C, E), dtype=np.float32)
    for b in range(B):
        out[b] = res.results[2 * b]["out"] + res.results[2 * b + 1]["out"]
    return out


# revision 27
# speedup vs baseline: 1.2554x; 1.2554x over previous
"""Multi-head causal attention (B=4, C=2048, E=1024, H=16, D=64) on 8 trn2 cores.

Sharding: core i = (batch b=i//2, head-group g=i%2).  Each core computes its
batch's attention for 8 heads (512 features) and a partial output projection;
the host sums the two partials per batch (W_o split row-wise).

Per-core kernel (all matmuls float32r: full PE rate at N>=256, FP22 operands):
  phase 1: V = x @ Wv_g            -> [tok, 8 heads x (64 feat + ones col)]
           QT/KT per head-pair     -> [128 feat, 2048 tok]   (x.T pre-done on host)
  phase 2: per (head-pair, q-chunk 512, k-block 128):
           S^T = K^T.T @ Q^T       (row-tiled pair, K=64 contraction x 2 heads)
           W^T = exp(S^T / 8)      (one ACT over both heads' psum banks)
           diagonal causal mask    (DVE multiply with host-provided mask)
           hid/rowsum = [V|1].T @ W^T  (M=65 accumulating over k-blocks)
           normalize by 1/rowsum -> hiddenT staged to DRAM
  phase 3: out = hiddenT.T @ Wo_g  (K=512 contraction via 4 chained matmuls)
"""

import numpy as np

import concourse.bass as bass
import concourse.mybir as mybir
import concourse.tile as tile
from concourse.vector_clock import ScopedClock

B, C, E = 4, 2048, 1024
H, D = 16, 64
N_CORES = 8
GF = 512          # features per head-group (8 heads x 64)
HP = 4            # head-pairs per group
QC = 512          # q-chunk width
KB = 128          # k-block width
NQC = C // QC     # 4
NKB = C // KB     # 16
NE = E // 128     # 8 contraction tiles over E
F32 = mybir.dt.float32
F32R = mybir.dt.float32r
BF16 = mybir.dt.bfloat16

_CACHED_NC = None


class PatchedTC(tile.TileContext):
    """This walrus build caps sync waits per instruction (1 for CTRL, ~2 for
    compute ISA structs).  Hoist excess waits onto same-engine NOPs emitted
    just before the instruction (engine streams execute in order, so the
    semantics are identical), and split the end-of-kernel drain's waits
    across single-wait drain instructions."""

    WAIT_CAP = 1

    def _commit_instruction(self, inst, lazy_reg_writes=True):
        si = getattr(inst, "sync_info", None)
        if (
            si is not None
            and len(si.on_wait) > self.WAIT_CAP
            and getattr(inst, "engine", mybir.EngineType.Unassigned)
            != mybir.EngineType.Unassigned
        ):
            waits = list(si.on_wait)
            keep = waits[: self.WAIT_CAP]
            extra = waits[self.WAIT_CAP :]
            si.on_wait[:] = keep
            for w in extra:
                nop = mybir.InstNoOp(
                    name=f"I-nw{self.nc.next_id()}",
                    engine=inst.engine,
                    bass_nofuse=True,
                    sync_info=mybir.SyncInfo(on_wait=[w], on_update=[]),
                )
                super()._commit_instruction(nop, lazy_reg_writes=False)
        return super()._commit_instruction(inst, lazy_reg_writes)

    def _drain_and_barrier(self, tick_clock, wait_clock):
        carrier = self.nc.sync.drain()
        wait_clock.add_sem_waits(
            carrier.ins, ScopedClock({None: tick_clock.global_clock})
        )
        si = carrier.ins.sync_info
        waits = list(si.on_wait) if si is not None else []
        if len(waits) > 1:
            si.on_wait[:] = waits[:1]
            for w in waits[1:]:
                extra = self.nc.sync.drain()
                extra.ins.sync_info = mybir.SyncInfo(on_wait=[w], on_update=[])
        self.nc.all_engine_barrier()
        assert self.sems is not None
        popped = self.nc._tile_sem_poison_stack.pop()
        assert popped is self._sem_poison
        self.nc.clear_and_free_semaphores(list(self.sems.allocated().values()))
        self.nc.all_engine_barrier()


def build_nc():
    nc = bass.Bass("TRN2", target_bir_lowering=False)
    xT = nc.declare_dram_parameter("xT", [E, C], BF16, isOutput=False)
    Wq = nc.declare_dram_parameter("Wq", [E, GF], BF16, isOutput=False)
    Wk = nc.declare_dram_parameter("Wk", [E, GF], BF16, isOutput=False)
    Wv = nc.declare_dram_parameter("Wv", [E, GF], BF16, isOutput=False)
    Wo = nc.declare_dram_parameter("Wo", [GF, E], BF16, isOutput=False)
    msk = nc.declare_dram_parameter("mask", [128, 4 * QC], mybir.dt.bfloat16, isOutput=False)
    out = nc.declare_dram_parameter("out", [C, E], F32, isOutput=True)

    xT_t = xT.ap().rearrange("(po pi) f -> pi po f", pi=128)    # [128, 8, C]
    Wq_t = Wq.ap().rearrange("(po pi) f -> pi po f", pi=128)    # [128, 8, GF]
    Wk_t = Wk.ap().rearrange("(po pi) f -> pi po f", pi=128)
    Wv_t = Wv.ap().rearrange("(po pi) f -> pi po f", pi=128)
    Wo_t = Wo.ap().rearrange("(po pi) f -> pi po f", pi=128)    # [128, 4, E]

    with PatchedTC(nc) as tc:
        import contextlib

        with contextlib.ExitStack() as ctx:
            consts = ctx.enter_context(tc.tile_pool(name="consts", bufs=1))
            dram = ctx.enter_context(tc.tile_pool(name="dram", bufs=1, space="DRAM"))
            ppsum = ctx.enter_context(tc.tile_pool(name="ppsum", bufs=2, space="PSUM"))

            mask_sb = consts.tile([128, 4 * QC], mybir.dt.bfloat16)
            nc.sync.dma_start(mask_sb[:], msk.ap())

            ctx2 = ctx.enter_context(contextlib.ExitStack())
            xpool = ctx2.enter_context(tc.tile_pool(name="xpool", bufs=1))
            vpool = ctx2.enter_context(tc.tile_pool(name="vpool", bufs=1))

            xT_sb = xpool.tile([128, NE, C], BF16)
            for e in range(NE):
                nc.sync.dma_start(xT_sb[:, e, :], xT_t[:, e, :])

            # ---- phase 1a: V for all 8 heads, ones column appended per head
            with tc.tile_pool(name="wvpool", bufs=1) as wvpool:
                wv_sb = wvpool.tile([128, NE, GF], BF16)
                nc.sync.dma_start(wv_sb[:], Wv_t[:])
                v_sb = vpool.tile([128, NKB, 2 * GF], BF16)  # [tok, kb, h*(64V|64ones)]
                nc.any.memset(v_sb[:], 1.0)
                for t in range(NKB):
                    pv = ppsum.tile([128, GF], F32, tag="ppsum")
                    for e in range(NE):
                        nc.tensor.matmul(
                            pv[:],
                            lhsT=xT_sb[:, e, t * 128 : (t + 1) * 128],
                            rhs=wv_sb[:, e, :],
                            start=(e == 0),
                            stop=(e == NE - 1),
                        )
                    dst = v_sb[:, t, :].rearrange("p (h u) -> p h u", u=128)[:, :, 0:64]
                    nc.vector.tensor_copy(dst, pv[:].rearrange("p (h u) -> p h u", u=64))

            # ---- phases 1b + 2: per head-pair projections + attention
            qkpool = ctx2.enter_context(tc.tile_pool(name="qkpool", bufs=2))
            wpool = ctx2.enter_context(tc.tile_pool(name="wpool", bufs=1))
            stpool = ctx2.enter_context(tc.tile_pool(name="stpsum", bufs=2, space="PSUM"))
            hidpool = ctx2.enter_context(tc.tile_pool(name="hidpsum", bufs=1, space="PSUM"))
            wtpool = ctx2.enter_context(tc.tile_pool(name="wtpool", bufs=2))
            napool = ctx2.enter_context(tc.tile_pool(name="napool", bufs=2))
            hidT_dram = dram.tile([HP, 128, C], BF16)

            for hp in range(HP):
                wq_sb = wpool.tile([128, NE, 128], BF16, tag="wq")
                wk_sb = wpool.tile([128, NE, 128], BF16, tag="wk")
                nc.sync.dma_start(wq_sb[:], Wq_t[:, :, hp * 128 : (hp + 1) * 128])
                nc.sync.dma_start(wk_sb[:], Wk_t[:, :, hp * 128 : (hp + 1) * 128])
                # fp32r here: bf16 row-tiled matmul pairs crash the exec unit
                # (NRT_EXEC_UNIT_UNRECOVERABLE); fp32r pairs are stable and the
                # 2 cyc/row fp32r rate over a concurrent pair matches unpaired
                # bf16 anyway.
                qt = qkpool.tile([128, C], F32R, tag="qt")
                kt = qkpool.tile([128, C], F32R, tag="kt")
                for n in range(NQC):
                    pq = ppsum.tile([128, QC], F32, tag="ppsum")
                    for e in range(NE):
                        nc.tensor.matmul(
                            pq[:],
                            lhsT=wq_sb[:, e, :],
                            rhs=xT_sb[:, e, n * QC : (n + 1) * QC],
                            start=(e == 0),
                            stop=(e == NE - 1),
                        )
                    nc.vector.tensor_copy(qt[:, n * QC : (n + 1) * QC], pq[:])
                    pk = ppsum.tile([128, QC], F32, tag="ppsum")
                    for e in range(NE):
                        nc.tensor.matmul(
                            pk[:],
                            lhsT=wk_sb[:, e, :],
                            rhs=xT_sb[:, e, n * QC : (n + 1) * QC],
                            start=(e == 0),
                            stop=(e == NE - 1),
                        )
                    nc.vector.tensor_copy(kt[:, n * QC : (n + 1) * QC], pk[:])

                for qc in range(NQC):
                    nkb = 4 * qc + 4
                    hidA = hidpool.tile([128, QC], F32, tag="hidA")
                    hidB = hidpool.tile([128, QC], F32, tag="hidB")
                    for kb in range(nkb):
                        st = stpool.tile([128, 2 * QC], F32, tag="st")
                        nc.tensor.matmul(
                            st[:, 0:QC],
                            lhsT=kt[0:64, kb * KB : (kb + 1) * KB],
                            rhs=qt[0:64, qc * QC : (qc + 1) * QC],
                            start=True,
                            stop=True,
                        )
                        nc.tensor.matmul(
                            st[:, QC : 2 * QC],
                            lhsT=kt[64:128, kb * KB : (kb + 1) * KB],
                            rhs=qt[64:128, qc * QC : (qc + 1) * QC],
                            start=True,
                            stop=True,
                        )
                        wt = wtpool.tile([128, 2 * QC], BF16, tag="wt")
                        nc.scalar.activation(
                            wt[:], st[:], mybir.ActivationFunctionType.Exp, scale=0.125
                        )
                        dr = kb - (nkb - 4)
                        if dr >= 0:
                            nc.vector.tensor_tensor(
                                wt[:].rearrange("p (a b) -> p a b", a=2),
                                wt[:].rearrange("p (a b) -> p a b", a=2),
                                mask_sb[:, None, dr * QC : (dr + 1) * QC].to_broadcast(
                                    (128, 2, QC)
                                ),
                                mybir.AluOpType.mult,
                            )
                        # hidden rows 0:64; rowsum replicated on rows 64:128
                        # (ones columns embedded in v_sb)
                        nc.tensor.matmul(
                            hidA[:],
                            lhsT=v_sb[:, kb, 2 * hp * 128 : (2 * hp + 1) * 128],
                            rhs=wt[:, 0:QC],
                            start=(kb == 0),
                            stop=(kb == nkb - 1),
                        )
                        nc.tensor.matmul(
                            hidB[:],
                            lhsT=v_sb[:, kb, (2 * hp + 1) * 128 : (2 * hp + 2) * 128],
                            rhs=wt[:, QC : 2 * QC],
                            start=(kb == 0),
                            stop=(kb == nkb - 1),
                        )
                    recA = napool.tile([64, QC], F32, tag="rec")
                    recB = napool.tile([64, QC], F32, tag="rec")
                    nc.vector.reciprocal(recA[:], hidA[64:128, :])
                    nc.vector.reciprocal(recB[:], hidB[64:128, :])
                    stage = napool.tile([128, QC], BF16, tag="stage")
                    nc.vector.tensor_tensor(
                        stage[0:64, :], hidA[0:64, :], recA[:], mybir.AluOpType.mult
                    )
                    nc.vector.tensor_tensor(
                        stage[64:128, :], hidB[0:64, :], recB[:], mybir.AluOpType.mult
                    )
                    nc.sync.dma_start(
                        hidT_dram[hp, :, qc * QC : (qc + 1) * QC], stage[:]
                    )

            # ---- phase 3: out projection, contracting all 512 group features
            ctx2.close()
            with tc.tile_pool(name="opool", bufs=1) as opool, tc.tile_pool(
                name="ostage", bufs=3
            ) as ostage:
                wo_sb = opool.tile([128, HP, E], BF16)
                nc.sync.dma_start(wo_sb[:], Wo_t[:])
                hf = opool.tile([128, HP, C], BF16)
                for f in range(HP):
                    nc.sync.dma_start(hf[:, f, :], hidT_dram[f, :, :])
                for qb in range(C // 128):
                    for ec in range(E // QC):
                        po = ppsum.tile([128, QC], F32, tag="ppsum")
                        for f in range(HP):
                            nc.tensor.matmul(
                                po[:],
                                lhsT=hf[:, f, qb * 128 : (qb + 1) * 128],
                                rhs=wo_sb[:, f, ec * QC : (ec + 1) * QC],
                                start=(f == 0),
                                stop=(f == HP - 1),
                            )
                        so = ostage.tile([128, QC], F32, tag="so")
                        nc.vector.tensor_copy(so[:], po[:])
                        nc.sync.dma_start(
                            out.ap()[qb * 128 : (qb + 1) * 128, ec * QC : (ec + 1) * QC],
                            so[:],
                        )
    return nc


def _make_mask():
    import ml_dtypes

    m = np.zeros((128, 4, QC), dtype=np.float32)
    for rr in range(4):
        kk = np.arange(128)[:, None]
        qq = np.arange(QC)[None, :]
        m[:, rr, :] = (128 * rr + kk <= qq).astype(np.float32)
    return np.ascontiguousarray(m.reshape(128, 4 * QC)).astype(ml_dtypes.bfloat16)


def make_in_maps(x, W_q, W_k, W_v, W_o):
    import ml_dtypes

    bf16 = ml_dtypes.bfloat16
    mask = _make_mask()
    in_maps = []
    for i in range(N_CORES):
        b, g = i // 2, i % 2
        in_maps.append(
            {
                "xT": np.ascontiguousarray(np.asarray(x)[b].T).astype(bf16),
                "Wq": np.ascontiguousarray(
                    np.asarray(W_q)[:, g * GF : (g + 1) * GF]
                ).astype(bf16),
                "Wk": np.ascontiguousarray(
                    np.asarray(W_k)[:, g * GF : (g + 1) * GF]
                ).astype(bf16),
                "Wv": np.ascontiguousarray(
                    np.asarray(W_v)[:, g * GF : (g + 1) * GF]
                ).astype(bf16),
                "Wo": np.ascontiguousarray(
                    np.asarray(W_o)[g * GF : (g + 1) * GF, :]
                ).astype(bf16),
                "mask": mask,
            }
        )
    return in_maps


def kernel(x, W_q, W_k, W_v, W_o):
    global _CACHED_NC
    from concourse.bass_utils import run_bass_kernel_spmd

    if _CACHED_NC is None:
        _CACHED_NC = build_nc()
    nc = _CACHED_NC

    in_maps = make_in_maps(x, W_q, W_k, W_v, W_o)
    res = run_bass_kernel_spmd(nc, in_maps, core_ids=list(range(N_CORES)))
    out = np.empty((B, C, E), dtype=np.float32)
    for b in range(B):
        out[b] = res.results[2 * b]["out"] + res.results[2 * b + 1]["out"]
    return out


# revision 29
# speedup vs baseline: 1.5822x; 1.2603x over previous
"""Multi-head causal attention (B=4, C=2048, E=1024, H=16, D=64) on 8 trn2 cores.

Sharding: core i = (batch b=i//2, head-group g=i%2).  Each core computes its
batch's attention for 8 heads (512 features) and a partial output projection;
the host sums the two partials per batch (W_o split row-wise).

Per-core kernel (all matmuls float32r: full PE rate at N>=256, FP22 operands):
  phase 1: V = x @ Wv_g            -> [tok, 8 heads x (64 feat + ones col)]
           QT/KT per head-pair     -> [128 feat, 2048 tok]   (x.T pre-done on host)
  phase 2: per (head-pair, q-chunk 512, k-block 128):
           S^T = K^T.T @ Q^T       (row-tiled pair, K=64 contraction x 2 heads)
           W^T = exp(S^T / 8)      (one ACT over both heads' psum banks)
           diagonal causal mask    (DVE multiply with host-provided mask)
           hid/rowsum = [V|1].T @ W^T  (M=65 accumulating over k-blocks)
           normalize by 1/rowsum -> hiddenT staged to DRAM
  phase 3: out = hiddenT.T @ Wo_g  (K=512 contraction via 4 chained matmuls)
"""

import numpy as np

import concourse.bass as bass
import concourse.mybir as mybir
import concourse.tile as tile
from concourse.vector_clock import ScopedClock

B, C, E = 4, 2048, 1024
H, D = 16, 64
N_CORES = 8
GF = 512          # features per head-group (8 heads x 64)
HP = 4            # head-pairs per group
QC = 512          # q-chunk width
KB = 128          # k-block width
NQC = C // QC     # 4
NKB = C // KB     # 16
NE = E // 128     # 8 contraction tiles over E
F32 = mybir.dt.float32
F32R = mybir.dt.float32r
BF16 = mybir.dt.bfloat16

_CACHED_NC = None


class PatchedTC(tile.TileContext):
    """This walrus build caps sync waits per instruction (1 for CTRL, ~2 for
    compute ISA structs).  Hoist excess waits onto same-engine NOPs emitted
    just before the instruction (engine streams execute in order, so the
    semantics are identical), and split the end-of-kernel drain's waits
    across single-wait drain instructions."""

    WAIT_CAP = 1

    def _commit_instruction(self, inst, lazy_reg_writes=True):
        si = getattr(inst, "sync_info", None)
        if (
            si is not None
            and len(si.on_wait) > self.WAIT_CAP
            and getattr(inst, "engine", mybir.EngineType.Unassigned)
            != mybir.EngineType.Unassigned
        ):
            waits = list(si.on_wait)
            keep = waits[: self.WAIT_CAP]
            extra = waits[self.WAIT_CAP :]
            si.on_wait[:] = keep
            for w in extra:
                nop = mybir.InstNoOp(
                    name=f"I-nw{self.nc.next_id()}",
                    engine=inst.engine,
                    bass_nofuse=True,
                    sync_info=mybir.SyncInfo(on_wait=[w], on_update=[]),
                )
                super()._commit_instruction(nop, lazy_reg_writes=False)
        return super()._commit_instruction(inst, lazy_reg_writes)

    def _drain_and_barrier(self, tick_clock, wait_clock):
        carrier = self.nc.sync.drain()
        wait_clock.add_sem_waits(
            carrier.ins, ScopedClock({None: tick_clock.global_clock})
        )
        si = carrier.ins.sync_info
        waits = list(si.on_wait) if si is not None else []
        if len(waits) > 1:
            si.on_wait[:] = waits[:1]
            for w in waits[1:]:
                extra = self.nc.sync.drain()
                extra.ins.sync_info = mybir.SyncInfo(on_wait=[w], on_update=[])
        self.nc.all_engine_barrier()
        assert self.sems is not None
        popped = self.nc._tile_sem_poison_stack.pop()
        assert popped is self._sem_poison
        self.nc.clear_and_free_semaphores(list(self.sems.allocated().values()))
        self.nc.all_engine_barrier()


def build_nc():
    nc = bass.Bass("TRN2", target_bir_lowering=False)
    xT = nc.declare_dram_parameter("xT", [E, C], BF16, isOutput=False)
    Wq = nc.declare_dram_parameter("Wq", [E, GF], BF16, isOutput=False)
    Wk = nc.declare_dram_parameter("Wk", [E, GF], BF16, isOutput=False)
    Wv = nc.declare_dram_parameter("Wv", [E, GF], BF16, isOutput=False)
    Wo = nc.declare_dram_parameter("Wo", [GF, E], BF16, isOutput=False)
    msk = nc.declare_dram_parameter("mask", [128, 4 * QC], mybir.dt.bfloat16, isOutput=False)
    out = nc.declare_dram_parameter("out", [C, E], F32, isOutput=True)

    xT_t = xT.ap().rearrange("(po pi) f -> pi po f", pi=128)    # [128, 8, C]
    Wq_t = Wq.ap().rearrange("(po pi) f -> pi po f", pi=128)    # [128, 8, GF]
    Wk_t = Wk.ap().rearrange("(po pi) f -> pi po f", pi=128)
    Wv_t = Wv.ap().rearrange("(po pi) f -> pi po f", pi=128)
    Wo_t = Wo.ap().rearrange("(po pi) f -> pi po f", pi=128)    # [128, 4, E]

    with PatchedTC(nc) as tc:
        import contextlib

        with contextlib.ExitStack() as ctx:
            consts = ctx.enter_context(tc.tile_pool(name="consts", bufs=1))
            dram = ctx.enter_context(tc.tile_pool(name="dram", bufs=1, space="DRAM"))
            ppsum = ctx.enter_context(tc.tile_pool(name="ppsum", bufs=2, space="PSUM"))

            mask_sb = consts.tile([128, 4 * QC], mybir.dt.bfloat16)
            nc.sync.dma_start(mask_sb[:], msk.ap())

            xpool = ctx.enter_context(tc.tile_pool(name="xpool", bufs=1))
            vpool = ctx.enter_context(tc.tile_pool(name="vpool", bufs=1))

            xT_sb = xpool.tile([128, NE, C], BF16)
            for e in range(NE):
                nc.sync.dma_start(xT_sb[:, e, :], xT_t[:, e, :])

            # ---- phase 1a: V for all 8 heads, ones column appended per head
            with tc.tile_pool(name="wvpool", bufs=1) as wvpool:
                wv_sb = wvpool.tile([128, NE, GF], BF16)
                nc.sync.dma_start(wv_sb[:], Wv_t[:])
                v_sb = vpool.tile([128, NKB, 2 * GF], BF16)  # [tok, kb, h*(64V|64ones)]
                nc.any.memset(v_sb[:], 1.0)
                for t in range(NKB):
                    pv = ppsum.tile([128, GF], F32, tag="ppsum")
                    for e in range(NE):
                        nc.tensor.matmul(
                            pv[:],
                            lhsT=xT_sb[:, e, t * 128 : (t + 1) * 128],
                            rhs=wv_sb[:, e, :],
                            start=(e == 0),
                            stop=(e == NE - 1),
                        )
                    dst = v_sb[:, t, :].rearrange("p (h u) -> p h u", u=128)[:, :, 0:64]
                    nc.vector.tensor_copy(dst, pv[:].rearrange("p (h u) -> p h u", u=64))

            # ---- phases 1b + 2: per head-pair projections + attention
            qkpool = ctx.enter_context(tc.tile_pool(name="qkpool", bufs=2))
            wpool = ctx.enter_context(tc.tile_pool(name="wpool", bufs=1))
            stpool = ctx.enter_context(tc.tile_pool(name="stpsum", bufs=2, space="PSUM"))
            hidpool = ctx.enter_context(tc.tile_pool(name="hidpsum", bufs=1, space="PSUM"))
            wtpool = ctx.enter_context(tc.tile_pool(name="wtpool", bufs=2))
            napool = ctx.enter_context(tc.tile_pool(name="napool", bufs=2))
            hidT_dram = dram.tile([HP, 128, C], BF16)

            for hp in range(HP):
                wq_sb = wpool.tile([128, NE, 128], BF16, tag="wq")
                wk_sb = wpool.tile([128, NE, 128], BF16, tag="wk")
                nc.sync.dma_start(wq_sb[:], Wq_t[:, :, hp * 128 : (hp + 1) * 128])
                nc.sync.dma_start(wk_sb[:], Wk_t[:, :, hp * 128 : (hp + 1) * 128])
                # fp32r here: bf16 row-tiled matmul pairs crash the exec unit
                # (NRT_EXEC_UNIT_UNRECOVERABLE); fp32r pairs are stable and the
                # 2 cyc/row fp32r rate over a concurrent pair matches unpaired
                # bf16 anyway.
                qt = qkpool.tile([128, C], F32R, tag="qt")
                kt = qkpool.tile([128, C], F32R, tag="kt")
                for n in range(NQC):
                    pq = ppsum.tile([128, QC], F32, tag="ppsum")
                    for e in range(NE):
                        nc.tensor.matmul(
                            pq[:],
                            lhsT=wq_sb[:, e, :],
                            rhs=xT_sb[:, e, n * QC : (n + 1) * QC],
                            start=(e == 0),
                            stop=(e == NE - 1),
                        )
                    nc.vector.tensor_copy(qt[:, n * QC : (n + 1) * QC], pq[:])
                    pk = ppsum.tile([128, QC], F32, tag="ppsum")
                    for e in range(NE):
                        nc.tensor.matmul(
                            pk[:],
                            lhsT=wk_sb[:, e, :],
                            rhs=xT_sb[:, e, n * QC : (n + 1) * QC],
                            start=(e == 0),
                            stop=(e == NE - 1),
                        )
                    nc.vector.tensor_copy(kt[:, n * QC : (n + 1) * QC], pk[:])

                for qc in range(NQC):
                    nkb = 4 * qc + 4
                    hidA = hidpool.tile([128, QC], F32, tag="hidA")
                    hidB = hidpool.tile([128, QC], F32, tag="hidB")
                    for kb in range(nkb):
                        st = stpool.tile([128, 2 * QC], F32, tag="st")
                        nc.tensor.matmul(
                            st[:, 0:QC],
                            lhsT=kt[0:64, kb * KB : (kb + 1) * KB],
                            rhs=qt[0:64, qc * QC : (qc + 1) * QC],
                            start=True,
                            stop=True,
                        )
                        nc.tensor.matmul(
                            st[:, QC : 2 * QC],
                            lhsT=kt[64:128, kb * KB : (kb + 1) * KB],
                            rhs=qt[64:128, qc * QC : (qc + 1) * QC],
                            start=True,
                            stop=True,
                        )
                        wt = wtpool.tile([128, 2 * QC], BF16, tag="wt")
                        nc.scalar.activation(
                            wt[:], st[:], mybir.ActivationFunctionType.Exp, scale=0.125
                        )
                        dr = kb - (nkb - 4)
                        if dr >= 0:
                            nc.vector.tensor_tensor(
                                wt[:].rearrange("p (a b) -> p a b", a=2),
                                wt[:].rearrange("p (a b) -> p a b", a=2),
                                mask_sb[:, None, dr * QC : (dr + 1) * QC].to_broadcast(
                                    (128, 2, QC)
                                ),
                                mybir.AluOpType.mult,
                            )
                        # hidden rows 0:64; rowsum replicated on rows 64:128
                        # (ones columns embedded in v_sb)
                        nc.tensor.matmul(
                            hidA[:],
                            lhsT=v_sb[:, kb, 2 * hp * 128 : (2 * hp + 1) * 128],
                            rhs=wt[:, 0:QC],
                            start=(kb == 0),
                            stop=(kb == nkb - 1),
                        )
                        nc.tensor.matmul(
                            hidB[:],
                            lhsT=v_sb[:, kb, (2 * hp + 1) * 128 : (2 * hp + 2) * 128],
                            rhs=wt[:, QC : 2 * QC],
                            start=(kb == 0),
                            stop=(kb == nkb - 1),
                        )
                    # 1/rowsum via exp(-ln(rs)) on ACT: DVE's bit-exact
                    # reciprocal is ~6 cycles/elem and custom DVE ops don't
                    # compile on this toolchain; ln/exp share one table set.
                    lnA = napool.tile([64, QC], F32, tag="ln")
                    lnB = napool.tile([64, QC], F32, tag="ln")
                    recA = napool.tile([64, QC], F32, tag="rec")
                    recB = napool.tile([64, QC], F32, tag="rec")
                    nc.scalar.activation(
                        lnA[:], hidA[64:128, :], mybir.ActivationFunctionType.Ln
                    )
                    nc.scalar.activation(
                        lnB[:], hidB[64:128, :], mybir.ActivationFunctionType.Ln
                    )
                    nc.scalar.activation(
                        recA[:], lnA[:], mybir.ActivationFunctionType.Exp, scale=-1.0
                    )
                    nc.scalar.activation(
                        recB[:], lnB[:], mybir.ActivationFunctionType.Exp, scale=-1.0
                    )
                    stage = napool.tile([128, QC], BF16, tag="stage")
                    nc.vector.tensor_tensor(
                        stage[0:64, :], hidA[0:64, :], recA[:], mybir.AluOpType.mult
                    )
                    nc.vector.tensor_tensor(
                        stage[64:128, :], hidB[0:64, :], recB[:], mybir.AluOpType.mult
                    )
                    nc.sync.dma_start(
                        hidT_dram[hp, :, qc * QC : (qc + 1) * QC], stage[:]
                    )

            # ---- phase 3: out projection, contracting all 512 group features
            with tc.tile_pool(name="opool", bufs=1) as opool, tc.tile_pool(
                name="ostage", bufs=3
            ) as ostage:
                wo_sb = opool.tile([128, HP, E], BF16)
                nc.sync.dma_start(wo_sb[:], Wo_t[:])
                hf = opool.tile([128, HP, C], BF16)
                for f in range(HP):
                    nc.sync.dma_start(hf[:, f, :], hidT_dram[f, :, :])
                for qb in range(C // 128):
                    for ec in range(E // QC):
                        po = ppsum.tile([128, QC], F32, tag="ppsum")
                        for f in range(HP):
                            nc.tensor.matmul(
                                po[:],
                                lhsT=hf[:, f, qb * 128 : (qb + 1) * 128],
                                rhs=wo_sb[:, f, ec * QC : (ec + 1) * QC],
                                start=(f == 0),
                                stop=(f == HP - 1),
                            )
                        so = ostage.tile([128, QC], F32, tag="so")
                        nc.vector.tensor_copy(so[:], po[:])
                        nc.sync.dma_start(
                            out.ap()[qb * 128 : (qb + 1) * 128, ec * QC : (ec + 1) * QC],
                            so[:],
                        )
    return nc


def _make_mask():
    import ml_dtypes

    m = np.zeros((128, 4, QC), dtype=np.float32)
    for rr in range(4):
        kk = np.arange(128)[:, None]
        qq = np.arange(QC)[None, :]
        m[:, rr, :] = (128 * rr + kk <= qq).astype(np.float32)
    return np.ascontiguousarray(m.reshape(128, 4 * QC)).astype(ml_dtypes.bfloat16)


def make_in_maps(x, W_q, W_k, W_v, W_o):
    import ml_dtypes

    bf16 = ml_dtypes.bfloat16
    mask = _make_mask()
    in_maps = []
    for i in range(N_CORES):
        b, g = i // 2, i % 2
        in_maps.append(
            {
                "xT": np.ascontiguousarray(np.asarray(x)[b].T).astype(bf16),
                "Wq": np.ascontiguousarray(
                    np.asarray(W_q)[:, g * GF : (g + 1) * GF]
                ).astype(bf16),
                "Wk": np.ascontiguousarray(
                    np.asarray(W_k)[:, g * GF : (g + 1) * GF]
                ).astype(bf16),
                "Wv": np.ascontiguousarray(
                    np.asarray(W_v)[:, g * GF : (g + 1) * GF]
                ).astype(bf16),
                "Wo": np.ascontiguousarray(
                    np.asarray(W_o)[g * GF : (g + 1) * GF, :]
                ).astype(bf16),
                "mask": mask,
            }
        )
    return in_maps


def kernel(x, W_q, W_k, W_v, W_o):
    global _CACHED_NC
    from concourse.bass_utils import run_bass_kernel_spmd

    if _CACHED_NC is None:
        _CACHED_NC = build_nc()
    nc = _CACHED_NC

    in_maps = make_in_maps(x, W_q, W_k, W_v, W_o)
    res = run_bass_kernel_spmd(nc, in_maps, core_ids=list(range(N_CORES)))
    out = np.empty((B, C, E), dtype=np.float32)
    for b in range(B):
        out[b] = res.results[2 * b]["out"] + res.results[2 * b + 1]["out"]
    return out
